# revision 26
# baseline (speedup 1.0000x reference)
"""CBOW negative-sampling loss kernel for 8 Trainium2 NeuronCores.

Math (faithful to the reference, including its [B]+[B,1] broadcast bug):
    c_b   = mean_w ctx_w[context[b, w]]               # [D]
    pos_b = log_sigmoid(emb_w[target[b]] . c_b)
    neg_b = sum_k log_sigmoid(emb_w[noise[b, k]] . c_b)
    out   = -(mean_b pos_b + mean_b neg_b) = -(sum_b (pos_b + neg_b)) / B

Strategy: shard B across the 8 cores (2048 samples each). Tables are cast to
bf16 on the host (halves gather traffic; the dots are ~1e-4 so bf16 costs
nothing against the fp32 reference envelope). Per core the host packs one
int32 index matrix; each group of blocks (128 samples each, group sizes
ramp 1,1,2,4,4,4 so compute starts early) issues indirect gathers:
  - ctx rows land one-row-per-partition, sample-major; the context mean is
    W accumulating TensorE matmuls against a static 0/1 pooling matrix
    (PSUM holds c in [sample, D(xblock)] layout), downcast to bf16 by ACT.
  - target+noise rows land K+1 segments per sample along partition p's free
    dim; DVE multiply against broadcast c (2x mode) + two pairwise folds
    (2x) + a short 1x reduce give all K+1 dots per sample.
Sigmoid runs per group on the scalar engine; one tail Ln with accum_out
yields per-partition summed log-sigmoid. Host sums per-core partials and
scales by -1/B.
"""

import numpy as np

V, D = 100000, 128
B, W, K = 16384, 10, 10
NCORES = 8
P = 128
B_LOCAL = B // NCORES  # 2048
NBLK = B_LOCAL // P  # 16 blocks of 128 samples
SCHEDULE = (1, 2, 4, 4, 4, 1)  # blocks per gather group (sums to NBLK)
SEG = W + 1 + K  # 21 rows gathered per sample
KP1 = K + 1

_LAST_RESULTS = None  # test harness introspection (exec_time_ns etc.)


def _build_bass(schedule, vocab):
    import concourse.bass as bass
    import concourse.tile as tile
    from concourse import bacc, mybir

    w = W
    nblk = sum(schedule)
    total_idx_cols = nblk * SEG
    pool_i32 = w * P // 2  # pool matrix bf16 cols viewed as int32
    nc = bacc.Bacc(None, target_bir_lowering=False)
    # idx columns + bitcast-packed bf16 pooling matrix in one input tensor
    # (one DMA + one completion wait at startup instead of two)
    meta_d = nc.declare_dram_parameter(
        "meta", [P, total_idx_cols + pool_i32], mybir.dt.int32, isOutput=False
    )
    ctx_w_d = nc.declare_dram_parameter(
        "ctx_w", [vocab, D], mybir.dt.bfloat16, isOutput=False
    )
    emb_w_d = nc.declare_dram_parameter(
        "emb_w", [vocab, D], mybir.dt.bfloat16, isOutput=False
    )
    out_d = nc.declare_dram_parameter("out", [P, 1], mybir.dt.float32, isOutput=True)

    with tile.TileContext(nc) as tc:
        with (
            tc.tile_pool(name="const", bufs=1) as cpool,
            tc.tile_pool(name="gather", bufs=4) as gpool,
            tc.tile_pool(name="work", bufs=3) as wpool,
            tc.tile_pool(name="psum", bufs=4, space="PSUM") as ppool,
        ):
            meta_sb = cpool.tile([P, total_idx_cols + pool_i32], mybir.dt.int32)
            nc.sync.dma_start(out=meta_sb[:], in_=meta_d[:])
            idx_sb = meta_sb[:, :total_idx_cols]
            pool_sb = meta_sb[:, total_idx_cols:].bitcast(mybir.dt.bfloat16)
            all_sig = cpool.tile([P, nblk * KP1], mybir.dt.float32)
            acc = cpool.tile([P, 1], mybir.dt.float32)

            c0 = 0  # idx column offset
            blk0 = 0  # block offset
            for gb in schedule:
                ctx_cols = gb * w
                emb_cols = gb * KP1
                # ctx rows, one per partition, sample-major: slot u*gb+b holds
                # group-rows (b*w*P + u*P) .. +127; split in two DMAs so the
                # first matmuls can start while the second half lands.
                Tctx = gpool.tile([P, ctx_cols * D], mybir.dt.bfloat16, tag="Tctx")
                # split the first group's ctx gather so its matmuls start
                # while the second half lands; later groups are one DMA
                # (each extra instruction costs ~1us of serialized DGE)
                splits = 2 if blk0 == 0 else 1
                cc = ctx_cols // splits
                for s in range(splits):
                    nc.gpsimd.indirect_dma_start(
                        out=Tctx[:, s * cc * D : (s + 1) * cc * D],
                        out_offset=None,
                        in_=ctx_w_d[:],
                        in_offset=bass.IndirectOffsetOnAxis(
                            ap=idx_sb[:, c0 + s * cc : c0 + (s + 1) * cc], axis=0
                        ),
                    )
                # target+noise rows: per block b, K+1 segments per sample
                # along partition p's free dim ([b][tgt, noise*K][D]).
                Temb = gpool.tile([P, emb_cols * D], mybir.dt.bfloat16, tag="Temb")
                nc.gpsimd.indirect_dma_start(
                    out=Temb[:],
                    out_offset=None,
                    in_=emb_w_d[:],
                    in_offset=bass.IndirectOffsetOnAxis(
                        ap=idx_sb[:, c0 + ctx_cols : c0 + ctx_cols + emb_cols],
                        axis=0,
                    ),
                )

                # c_raw[s, d(+block)] = sum_r pool[r, s] * ctx_rows[r, d] on
                # TensorE; slot u holds all gb blocks' tiles side by side so
                # one gb*128-wide matmul per u covers the whole group.
                c_ps = ppool.tile([P, gb * D], mybir.dt.float32, tag="cps")
                for u in range(w):
                    nc.tensor.matmul(
                        c_ps[:],
                        lhsT=pool_sb[:, u * P : (u + 1) * P],
                        rhs=Tctx[:, u * gb * D : (u + 1) * gb * D],
                        start=(u == 0),
                        stop=(u == w - 1),
                    )
                c_sb = wpool.tile([P, gb * D], mybir.dt.bfloat16, tag="c")
                nc.scalar.activation(
                    out=c_sb[:],
                    in_=c_ps[:],
                    func=mybir.ActivationFunctionType.Copy,
                )

                # all (K+1)*gb dots: multiply (2x mode), two pairwise folds
                # (2x), then a short 1x reduce (TENSOR_REDUCE has no 2x uop).
                seg = gb * KP1
                prod = wpool.tile([P, seg * D], mybir.dt.bfloat16, tag="prod")
                nc.vector.tensor_tensor(
                    out=prod[:],
                    in0=Temb[:],
                    in1=c_sb[:]
                    .rearrange("p (b d) -> p b d", b=gb)
                    .unsqueeze(2)
                    .broadcast_to([P, gb, KP1, D]),
                    op=mybir.AluOpType.mult,
                )
                pv = prod[:].rearrange("p (s h d) -> p s h d", s=seg, h=2)
                fold = wpool.tile([P, seg * D // 2], mybir.dt.bfloat16, tag="fold")
                nc.vector.tensor_tensor(
                    out=fold[:],
                    in0=pv[:, :, 0, :],
                    in1=pv[:, :, 1, :],
                    op=mybir.AluOpType.add,
                )
                fv = fold[:].rearrange("p (s h d) -> p s h d", s=seg, h=2)
                fold2 = wpool.tile([P, seg * D // 4], mybir.dt.bfloat16, tag="fold2")
                nc.vector.tensor_tensor(
                    out=fold2[:],
                    in0=fv[:, :, 0, :],
                    in1=fv[:, :, 1, :],
                    op=mybir.AluOpType.add,
                )
                dots = wpool.tile([P, seg], mybir.dt.bfloat16, tag="dots")
                with nc.allow_low_precision("dots are ~1e-4; bf16 keeps DVE 2x"):
                    nc.vector.tensor_reduce(
                        out=dots[:],
                        in_=fold2[:].rearrange("p (s d) -> p s d", s=seg),
                        axis=mybir.AxisListType.X,
                        op=mybir.AluOpType.add,
                    )
                # sigmoid of true dots per group (1/W rescales ctx sum to
                # mean); off the critical path on the scalar engine.
                nc.scalar.activation(
                    out=all_sig[:, blk0 * KP1 : (blk0 + gb) * KP1],
                    in_=dots[:],
                    func=mybir.ActivationFunctionType.Sigmoid,
                    scale=1.0 / w,
                )
                c0 += ctx_cols + emb_cols
                blk0 += gb

            # One tail Ln; accum_out emits per-partition sum of log-sigmoids.
            ls = cpool.tile([P, nblk * KP1], mybir.dt.float32)
            nc.scalar.activation(
                out=ls[:],
                in_=all_sig[:],
                func=mybir.ActivationFunctionType.Ln,
                accum_out=acc[:, 0:1],
            )
            nc.sync.dma_start(out=out_d[:], in_=acc[:])
    nc.compile()
    return nc


def _make_pool_matrix():
    """[P, W*P] bf16: pool[r, u*P + s] = 1 iff row u*128+r belongs to sample s."""
    import ml_dtypes

    pool = np.zeros((P, W * P), dtype=np.float32)
    for u in range(W):
        for r in range(P):
            s = (u * P + r) // W  # sample-in-block, < 128
            pool[r, u * P + s] = 1.0
    return pool.astype(ml_dtypes.bfloat16)


def _pack_indices(context, target, noise, ncores, schedule):
    """Per-core [P, nblk*SEG] int32 index matrices in gather layout."""
    nblk = sum(schedule)
    ctx_r = np.ascontiguousarray(context, dtype=np.int32).reshape(ncores, nblk, P, W)
    tgt_r = np.ascontiguousarray(target, dtype=np.int32).reshape(ncores, nblk, P)
    noi_r = np.ascontiguousarray(noise, dtype=np.int32).reshape(ncores, nblk, P, K)
    idxs = []
    for n in range(ncores):
        cols = []
        b0 = 0
        for gb in schedule:
            spg = gb * P
            # ctx: slot u*gb+b holds group-rows b*W*P + u*P + p (blocks side
            # by side per pooling slot u)
            flat = ctx_r[n, b0 : b0 + gb].reshape(spg * W)  # (sample, word)
            ctx_part = flat.reshape(gb, W, P).transpose(1, 0, 2).reshape(gb * W, P).T
            # emb: per block, [tgt, noise*K] per sample
            emb_part = np.concatenate(
                [
                    np.concatenate(
                        [tgt_r[n, b0 + b][:, None], noi_r[n, b0 + b]], axis=1
                    )
                    for b in range(gb)
                ],
                axis=1,
            )  # [P, gb*(K+1)]
            cols.append(np.concatenate([ctx_part, emb_part], axis=1))
            b0 += gb
        idxs.append(np.ascontiguousarray(np.concatenate(cols, axis=1)))
    return idxs


def kernel(context, target, noise, emb_w, ctx_w):
    global _LAST_RESULTS
    import os
    import sys

    for p in ("/root/.axon_site/_ro/trn_rl_repo", "/opt/trn_rl_repo"):
        if p not in sys.path:
            sys.path.insert(0, p)
    import ml_dtypes

    from concourse.bass_utils import run_bass_kernel_spmd

    context = np.asarray(context)
    target = np.asarray(target)
    noise = np.asarray(noise)
    bf16 = ml_dtypes.bfloat16
    emb_w = np.ascontiguousarray(np.asarray(emb_w, dtype=np.float32).astype(bf16))
    ctx_w = np.ascontiguousarray(np.asarray(ctx_w, dtype=np.float32).astype(bf16))

    nc = _build_bass(SCHEDULE, V)
    idxs = _pack_indices(context, target, noise, NCORES, SCHEDULE)
    pool_i32 = np.ascontiguousarray(_make_pool_matrix()).view(np.int32)
    in_maps = [
        {
            "meta": np.ascontiguousarray(
                np.concatenate([idxs[n], pool_i32], axis=1)
            ),
            "ctx_w": ctx_w,
            "emb_w": emb_w,
        }
        for n in range(NCORES)
    ]
    tmpdir = os.environ.get("KERNEL_TMPDIR") or None
    res = run_bass_kernel_spmd(nc, in_maps, list(range(NCORES)), tmpdir=tmpdir)
    _LAST_RESULTS = res
    total = sum(
        float(np.sum(np.asarray(r["out"], dtype=np.float64))) for r in res.results
    )
    return np.float32(-total / B)


# revision 27
# speedup vs baseline: 1.0598x; 1.0598x over previous
"""CBOW negative-sampling loss kernel for 8 Trainium2 NeuronCores.

Math (faithful to the reference, including its [B]+[B,1] broadcast bug):
    c_b   = mean_w ctx_w[context[b, w]]               # [D]
    pos_b = log_sigmoid(emb_w[target[b]] . c_b)
    neg_b = sum_k log_sigmoid(emb_w[noise[b, k]] . c_b)
    out   = -(mean_b pos_b + mean_b neg_b) = -(sum_b (pos_b + neg_b)) / B

Strategy: shard B across the 8 cores (2048 samples each). Tables are cast to
bf16 on the host (halves the random-gather HBM traffic; the dots are ~1e-4
so bf16 quantization is far inside the fp32 reference envelope). Per core
the host packs one int32 index matrix; each 256-sample group issues two
indirect (gather) DMAs:
  - ctx rows land one-row-per-partition, sample-major, with the two blocks'
    tiles side by side per pooling slot, so the context mean is 10
    accumulating 256-wide TensorE matmuls against a static 0/1 pooling
    matrix (PSUM holds c in [sample, D x block] layout); the scalar engine
    downcasts c to bf16.
  - target+noise rows land 11 segments per sample along partition p's free
    dim; per block, one DVE multiply against broadcast c (bf16 2x mode) +
    one strided reduce gives all 11 dots per sample.
One tail Sigmoid(0.1*x) + Ln pass on the scalar engine (Ln's accum_out)
yields per-partition summed log-sigmoid. The host sums the per-core partials
and scales by -1/B.
"""

import numpy as np

V, D = 100000, 128
B, W, K = 16384, 10, 10
NCORES = 8
P = 128
B_LOCAL = B // NCORES  # 2048
NBLK = B_LOCAL // P  # 16 blocks of 128 samples
GB = 2  # blocks per gather group
NGRP = NBLK // GB  # 8 groups
SEG = W + 1 + K  # 21 rows gathered per sample
CTX_COLS = GB * W  # 20 ctx gather slots per group
EMB_COLS = GB * (K + 1)  # 22 emb gather slots per group
GSEG = CTX_COLS + EMB_COLS  # 42 index columns per group

_LAST_RESULTS = None  # test harness introspection (exec_time_ns etc.)


def _build_bass(ngrp, gb, vocab):
    import concourse.bass as bass
    import concourse.tile as tile
    from concourse import bacc, mybir

    w, k = W, K
    kp1 = k + 1
    ctx_cols = gb * w
    emb_cols = gb * kp1
    gseg = ctx_cols + emb_cols
    nc = bacc.Bacc(None, target_bir_lowering=False)
    idx_d = nc.declare_dram_parameter(
        "idx", [P, ngrp * gseg], mybir.dt.int32, isOutput=False
    )
    pool_d = nc.declare_dram_parameter(
        "pool", [P, w * P], mybir.dt.bfloat16, isOutput=False
    )
    ctx_w_d = nc.declare_dram_parameter(
        "ctx_w", [vocab, D], mybir.dt.bfloat16, isOutput=False
    )
    emb_w_d = nc.declare_dram_parameter(
        "emb_w", [vocab, D], mybir.dt.bfloat16, isOutput=False
    )
    out_d = nc.declare_dram_parameter("out", [P, 1], mybir.dt.float32, isOutput=True)

    with tile.TileContext(nc) as tc:
        with (
            tc.tile_pool(name="const", bufs=1) as cpool,
            tc.tile_pool(name="gather", bufs=4) as gpool,
            tc.tile_pool(name="work", bufs=3) as wpool,
            tc.tile_pool(name="psum", bufs=4, space="PSUM") as ppool,
        ):
            idx_sb = cpool.tile([P, ngrp * gseg], mybir.dt.int32)
            nc.sync.dma_start(out=idx_sb[:], in_=idx_d[:])
            pool_sb = cpool.tile([P, w * P], mybir.dt.bfloat16)
            nc.sync.dma_start(out=pool_sb[:], in_=pool_d[:])
            all_dots = cpool.tile([P, ngrp * gb * kp1], mybir.dt.float32)
            acc = cpool.tile([P, 1], mybir.dt.float32)

            for g in range(ngrp):
                c0 = g * gseg
                # ctx rows, one per partition, sample-major: slot u*gb+b holds
                # group-rows b*W*128 + u*128 .. +127 (blocks side by side per
                # pooling slot u).
                Tctx = gpool.tile([P, ctx_cols * D], mybir.dt.bfloat16, tag="Tctx")
                nc.gpsimd.indirect_dma_start(
                    out=Tctx[:],
                    out_offset=None,
                    in_=ctx_w_d[:],
                    in_offset=bass.IndirectOffsetOnAxis(
                        ap=idx_sb[:, c0 : c0 + ctx_cols], axis=0
                    ),
                )
                # target+noise rows: per block b, 11 segments per sample along
                # partition p's free dim ([b][tgt, noise*10][D]).
                Temb = gpool.tile([P, emb_cols * D], mybir.dt.bfloat16, tag="Temb")
                nc.gpsimd.indirect_dma_start(
                    out=Temb[:],
                    out_offset=None,
                    in_=emb_w_d[:],
                    in_offset=bass.IndirectOffsetOnAxis(
                        ap=idx_sb[:, c0 + ctx_cols : c0 + gseg], axis=0
                    ),
                )

                c_sb = wpool.tile([P, gb * D], mybir.dt.bfloat16, tag="c")
                dots = all_dots[:, g * gb * kp1 : (g + 1) * gb * kp1]
                # c_raw[s, d(+block)] = sum_r pool[r, s] * ctx_rows[r, d] on
                # TensorE; slot u holds both blocks' tiles side by side so one
                # 256-wide matmul per u covers the whole group.
                c_ps = ppool.tile([P, gb * D], mybir.dt.float32, tag="cps")
                for u in range(w):
                    nc.tensor.matmul(
                        c_ps[:],
                        lhsT=pool_sb[:, u * P : (u + 1) * P],
                        rhs=Tctx[:, u * gb * D : (u + 1) * gb * D],
                        start=(u == 0),
                        stop=(u == w - 1),
                    )
                nc.scalar.activation(
                    out=c_sb[:],
                    in_=c_ps[:],
                    func=mybir.ActivationFunctionType.Copy,
                )
                for b in range(gb):
                    # all 11 dots for block b in one multiply + one reduce
                    prod = wpool.tile([P, kp1 * D], mybir.dt.bfloat16, tag="prod")
                    nc.vector.tensor_tensor(
                        out=prod[:],
                        in0=Temb[:, b * kp1 * D : (b + 1) * kp1 * D],
                        in1=c_sb[:, b * D : (b + 1) * D]
                        .unsqueeze(1)
                        .broadcast_to([P, kp1, D]),
                        op=mybir.AluOpType.mult,
                    )
                    nc.vector.tensor_reduce(
                        out=dots[:, b * kp1 : (b + 1) * kp1],
                        in_=prod[:].rearrange("p (s d) -> p s d", s=kp1),
                        axis=mybir.AxisListType.X,
                        op=mybir.AluOpType.add,
                    )

            # One tail pass: log-sigmoid of all true dots (0.1 rescales the
            # ctx sum to a mean); Ln's accum_out emits per-partition sums.
            sig = cpool.tile([P, ngrp * gb * kp1], mybir.dt.float32)
            nc.scalar.activation(
                out=sig[:],
                in_=all_dots[:],
                func=mybir.ActivationFunctionType.Sigmoid,
                scale=1.0 / w,
            )
            ls = cpool.tile([P, ngrp * gb * kp1], mybir.dt.float32)
            nc.scalar.activation(
                out=ls[:],
                in_=sig[:],
                func=mybir.ActivationFunctionType.Ln,
                accum_out=acc[:, 0:1],
            )

            nc.sync.dma_start(out=out_d[:], in_=acc[:])
    nc.compile()
    return nc


def _make_pool_matrix():
    """[P, W*P] bf16: pool[r, u*P + s] = 1 iff row u*128+r belongs to sample s."""
    import ml_dtypes

    pool = np.zeros((P, W * P), dtype=np.float32)
    for u in range(W):
        for r in range(P):
            s = (u * P + r) // W  # sample-in-block, < 128
            pool[r, u * P + s] = 1.0
    return pool.astype(ml_dtypes.bfloat16)


def _pack_indices(context, target, noise, ncores, nblk, gb):
    """Per-core [P, ngrp*GSEG] int32 index matrices in gather layout."""
    ngrp = nblk // gb
    spg = gb * P  # samples per group
    ctx_cols = gb * W
    ctx_r = np.ascontiguousarray(context, dtype=np.int32).reshape(ncores, ngrp, spg, W)
    tgt_r = np.ascontiguousarray(target, dtype=np.int32).reshape(ncores, ngrp, gb, P)
    noi_r = np.ascontiguousarray(noise, dtype=np.int32).reshape(ncores, ngrp, gb, P, K)
    idxs = []
    for n in range(ncores):
        cols = []
        for g in range(ngrp):
            # ctx: slot u*gb+b holds group-rows b*W*128 + u*128 + p (so both
            # blocks' tiles for pooling-slot u sit side by side)
            flat = ctx_r[n, g].reshape(spg * W)  # ordered (sample, word)
            ctx_part = (
                flat.reshape(gb, W, P).transpose(1, 0, 2).reshape(ctx_cols, P).T
            )
            # emb: per block, [tgt, noise*10] per sample
            emb_part = np.concatenate(
                [
                    np.concatenate(
                        [tgt_r[n, g, b][:, None], noi_r[n, g, b]], axis=1
                    )  # [P, 11]
                    for b in range(gb)
                ],
                axis=1,
            )  # [P, gb*11]
            cols.append(np.concatenate([ctx_part, emb_part], axis=1))
        idxs.append(np.ascontiguousarray(np.concatenate(cols, axis=1)))
    return idxs


def kernel(context, target, noise, emb_w, ctx_w):
    global _LAST_RESULTS
    import os
    import sys

    for p in ("/root/.axon_site/_ro/trn_rl_repo", "/opt/trn_rl_repo"):
        if p not in sys.path:
            sys.path.insert(0, p)
    import ml_dtypes

    from concourse.bass_utils import run_bass_kernel_spmd

    context = np.asarray(context)
    target = np.asarray(target)
    noise = np.asarray(noise)
    bf16 = ml_dtypes.bfloat16
    emb_w = np.ascontiguousarray(np.asarray(emb_w, dtype=np.float32).astype(bf16))
    ctx_w = np.ascontiguousarray(np.asarray(ctx_w, dtype=np.float32).astype(bf16))

    nc = _build_bass(NGRP, GB, V)
    idxs = _pack_indices(context, target, noise, NCORES, NBLK, GB)
    pool = _make_pool_matrix()
    in_maps = [
        {"idx": idxs[n], "pool": pool, "ctx_w": ctx_w, "emb_w": emb_w}
        for n in range(NCORES)
    ]
    tmpdir = os.environ.get("KERNEL_TMPDIR") or None
    res = run_bass_kernel_spmd(nc, in_maps, list(range(NCORES)), tmpdir=tmpdir)
    _LAST_RESULTS = res
    total = sum(
        float(np.sum(np.asarray(r["out"], dtype=np.float64))) for r in res.results
    )
    return np.float32(-total / B)
